# revision 1
# baseline (speedup 1.0000x reference)
"""Causal group-query attention on 8 trn2 NeuronCores.

Sharding: 2 batches x 4 KV-head groups = 8 cores. Each core computes, for its
(batch b, group g): q/k/v projections for the group's 4 query heads + 1 KV
head, causal attention, and a partial output projection against the group's
512 rows of wo. Host sums the 4 group partials per batch.

On-chip layout (all fp16 on the PE, fp32 PSUM accumulation):
  xT  [128(Hp), 16(Ho), T]   <- DMA-transpose of host-cast x16
  qT  [128(d), 4(h), T]      <- wq.T @ xT   (per-head slices)
  kT  [128(d), T]            <- wk.T @ xT
  v   [128(Ts), 16(tb), 128(d)] natural    <- xT.T @ wv
  scoresT [Ts=128, Tq<=512] = kT_slice.T @ qT_slice    (PSUM f32)
  probsT  = exp(scale*scoresT + causal mask)           (ACT -> fp16)
  oT  [128(d), Tq] += v_s.T @ probsT ; denom[1, Tq] += ones.T @ probsT
  out partial = (oT/denom).T @ wo_shard                (PSUM f32 -> DRAM)
"""

import numpy as np

import concourse.bass as bass
import concourse.mybir as mybir
import concourse.tile as tile
from concourse import bacc
from concourse.bass_utils import run_bass_kernel_spmd

# Problem shapes (hardcoded per contract)
B = 2
T = 2048
H = 2048
NH = 16
NKV = 4
D = 128  # head dim
G = NKV  # groups = cores per batch
HPG = NH // NKV  # 4 query heads per group
DQ = HPG * D  # 512 q dims per group
P = 128
KO = H // P  # 16 contraction subtiles
TCH = 512  # T chunk
NCH = T // TCH  # 4
NT = T // P  # 16
F16 = mybir.dt.float16
F32 = mybir.dt.float32
SCALE = float(1.0 / np.sqrt(D))
MASK_NEG = -1.0e9

AF = mybir.ActivationFunctionType
ALU = mybir.AluOpType


def build_nc(reps: int = 1):
    nc = bacc.Bacc(
        "TRN2",
        target_bir_lowering=False,
        debug=False,
        enable_asserts=False,
        num_devices=8,
    )
    x16 = nc.dram_tensor("x16", [T, H], F16, kind="ExternalInput").ap()
    wq16 = nc.dram_tensor("wq16", [P, KO, DQ], F16, kind="ExternalInput").ap()
    wk16 = nc.dram_tensor("wk16", [P, KO, D], F16, kind="ExternalInput").ap()
    wv16 = nc.dram_tensor("wv16", [P, KO, D], F16, kind="ExternalInput").ap()
    wo16 = nc.dram_tensor("wo16", [P, HPG, H], F16, kind="ExternalInput").ap()
    bq16 = nc.dram_tensor("bq16", [P, HPG], F16, kind="ExternalInput").ap()
    onem = nc.dram_tensor("onem", [P, P], F16, kind="ExternalInput").ap()
    zmask = nc.dram_tensor("zmask", [P, P], F16, kind="ExternalInput").ap()
    out = nc.dram_tensor("out", [T, H], F32, kind="ExternalOutput").ap()

    with tile.TileContext(nc) as tc:
        with (
            tc.tile_pool(name="const", bufs=1) as cp,
            tc.tile_pool(name="pers", bufs=1) as pp,
            tc.tile_pool(name="probs", bufs=8) as prp,
            tc.tile_pool(name="bcast", bufs=3) as bcp,
            tc.tile_pool(name="outb", bufs=4) as obp,
            tc.tile_pool(name="mmps", bufs=2, space="PSUM") as mm_ps,
            tc.tile_pool(name="scps", bufs=4, space="PSUM") as sc_psp,
            tc.tile_pool(name="otps", bufs=1, space="PSUM") as ot_psp,
            tc.tile_pool(name="denps", bufs=1, space="PSUM") as den_psp,
        ):
            # ---- persistent SBUF residents ----
            wq_sb = cp.tile([P, KO, DQ], F16)
            wk_sb = cp.tile([P, KO, D], F16)
            wv_sb = cp.tile([P, KO, D], F16)
            wo_sb = cp.tile([P, HPG, H], F16)
            bq_sb = cp.tile([P, HPG], F16)
            onem_sb = cp.tile([P, P], F16)
            zmask_sb = cp.tile([P, P], F16)

            xT_c = [pp.tile([P, KO, TCH], F16, name=f"xT{c}") for c in range(NCH)]
            qT_t = [
                [pp.tile([P, TCH], F16, name=f"qT{h}_{c}") for c in range(NCH)]
                for h in range(HPG)
            ]
            kT_t = [pp.tile([P, TCH], F16, name=f"kT{c}") for c in range(NCH)]
            v_t = [pp.tile([P, D], F16, name=f"v{tb}") for tb in range(NT)]
            oT_t = [
                [pp.tile([P, TCH], F16, name=f"oT{h}_{c}") for c in range(NCH)]
                for h in range(HPG)
            ]

            def transpose_chunk(c):
                for tl in range(4):
                    tb = c * 4 + tl
                    nc.sync.dma_start_transpose(
                        xT_c[c][:, :, tl * P : (tl + 1) * P],
                        x16[tb * P : (tb + 1) * P, :],
                    )

            def body(first=False):
                if not first:
                    transpose_chunk(0)
                for c in range(NCH):
                    if c + 1 < NCH:
                        transpose_chunk(c + 1)
                    xc = xT_c[c]
                    # ---- projections for chunk c ----
                    for m in range(HPG):
                        ps = mm_ps.tile([P, TCH], F32, name="mm", tag="mm")
                        for k in range(KO):
                            nc.tensor.matmul(
                                ps[:],
                                wq_sb[:, k, m * P : (m + 1) * P],
                                xc[:, k, :],
                                start=(k == 0),
                                stop=(k == KO - 1),
                            )
                        nc.vector.tensor_tensor(
                            qT_t[m][c][:],
                            ps[:],
                            bq_sb[:, m : m + 1].to_broadcast((P, TCH)),
                            ALU.add,
                        )
                    ps = mm_ps.tile([P, TCH], F32, name="mm", tag="mm")
                    for k in range(KO):
                        nc.tensor.matmul(
                            ps[:],
                            wk_sb[:, k, :],
                            xc[:, k, :],
                            start=(k == 0),
                            stop=(k == KO - 1),
                        )
                    nc.vector.tensor_copy(kT_t[c][:], ps[:])
                    for tl in range(4):
                        tb = c * 4 + tl
                        ps = mm_ps.tile([P, TCH], F32, name="mm", tag="mm")
                        for k in range(KO):
                            nc.tensor.matmul(
                                ps[:, :D],
                                xc[:, k, tl * P : (tl + 1) * P],
                                wv_sb[:, k, :],
                                start=(k == 0),
                                stop=(k == KO - 1),
                            )
                        nc.vector.tensor_copy(v_t[tb][:], ps[:, :D])

                    # ---- attention for all heads, q-chunk c ----
                    for h in range(HPG):
                        oT_ps = ot_psp.tile([P, TCH], F32, name="oT_ps", tag="oT")
                        den_ps = den_psp.tile([P, TCH], F32, name="den_ps", tag="den")
                        nsb = 4 * c + 4
                        for j in range(nsb):
                            jj = j - 4 * c
                            lo = max(0, jj) * P
                            sc = sc_psp.tile([P, TCH], F32, name="sc", tag="sc")
                            nc.tensor.matmul(
                                sc[:, lo:],
                                kT_t[j // 4][:, (j % 4) * P : (j % 4 + 1) * P],
                                qT_t[h][c][:, lo:],
                                start=True,
                                stop=True,
                            )
                            pr = prp.tile([P, TCH], F16, name="pr", tag="pr")
                            nc.scalar.activation(
                                pr[:, lo:], sc[:, lo:], AF.Exp, scale=SCALE
                            )
                            if jj >= 0:
                                nc.vector.tensor_tensor(
                                    pr[:, jj * P : (jj + 1) * P],
                                    pr[:, jj * P : (jj + 1) * P],
                                    zmask_sb[:],
                                    ALU.mult,
                                )
                            nc.tensor.matmul(
                                den_ps[:, lo:],
                                onem_sb[:],
                                pr[:, lo:],
                                start=(j == 0),
                                stop=(j == nsb - 1),
                                skip_group_check=True,
                            )
                            nc.tensor.matmul(
                                oT_ps[:, lo:],
                                v_t[j][:],
                                pr[:, lo:],
                                start=(j == 0),
                                stop=(j == nsb - 1),
                                skip_group_check=True,
                            )
                        bc32 = bcp.tile([P, TCH], F32, name="bc32", tag="bc")
                        nc.vector.reciprocal(bc32[:], den_ps[:])
                        nc.vector.tensor_tensor(
                            oT_t[h][c][:], oT_ps[:], bc32[:], ALU.mult
                        )

                    # ---- output projection for chunk c rows ----
                    for tl in range(4):
                        tb = c * 4 + tl
                        for n in range(NCH):
                            ps = mm_ps.tile([P, TCH], F32, name="mm", tag="mm")
                            for hh in range(HPG):
                                nc.tensor.matmul(
                                    ps[:],
                                    oT_t[hh][c][:, tl * P : (tl + 1) * P],
                                    wo_sb[:, hh, n * TCH : (n + 1) * TCH],
                                    start=(hh == 0),
                                    stop=(hh == HPG - 1),
                                )
                            ob = obp.tile([P, TCH], F32, name="ob", tag="ob")
                            nc.vector.tensor_copy(ob[:], ps[:])
                            nc.sync.dma_start(
                                out[tb * P : (tb + 1) * P, n * TCH : (n + 1) * TCH],
                                ob[:],
                            )

            transpose_chunk(0)
            nc.sync.dma_start(wq_sb[:], wq16)
            nc.sync.dma_start(wk_sb[:], wk16)
            nc.sync.dma_start(wv_sb[:], wv16)
            nc.sync.dma_start(bq_sb[:], bq16)
            nc.sync.dma_start(onem_sb[:], onem)
            nc.sync.dma_start(zmask_sb[:], zmask)
            nc.sync.dma_start(wo_sb[:], wo16)
            if reps == 1:
                body(first=True)
            else:
                body(first=True)
                with tc.For_i(0, reps - 1, 1):
                    body()

    nc.compile()
    return nc


def make_in_maps(x, wq, bq, wk, bk, wv, bv, wo):
    # bk shifts every score in a query row equally (softmax-invariant) and is
    # dropped; bv passes through softmax as a constant row handled on host.
    del bk, bv
    f16 = np.float16
    zmask = np.zeros((P, P), f16)
    for s in range(P):
        zmask[s, s:] = 1.0  # keep q >= s
    onem = np.ones((P, P), f16)
    in_maps = []
    for core in range(8):
        b, g = divmod(core, G)
        wq_s = wq[:, g * DQ : (g + 1) * DQ].astype(f16)
        wk_s = wk[:, g * D : (g + 1) * D].astype(f16)
        wv_s = wv[:, g * D : (g + 1) * D].astype(f16)
        wo_s = wo[g * DQ : (g + 1) * DQ, :].astype(f16)
        in_maps.append(
            {
                "x16": np.ascontiguousarray(x[b].astype(f16)),
                "wq16": np.ascontiguousarray(
                    wq_s.reshape(KO, P, DQ).transpose(1, 0, 2)
                ),
                "wk16": np.ascontiguousarray(wk_s.reshape(KO, P, D).transpose(1, 0, 2)),
                "wv16": np.ascontiguousarray(wv_s.reshape(KO, P, D).transpose(1, 0, 2)),
                "wo16": np.ascontiguousarray(
                    wo_s.reshape(HPG, P, H).transpose(1, 0, 2)
                ),
                "bq16": np.ascontiguousarray(
                    bq[g * DQ : (g + 1) * DQ].astype(f16).reshape(HPG, P).T
                ),
                "onem": onem,
                "zmask": zmask,
            }
        )
    return in_maps


_NC_CACHE = {}


def get_nc(reps: int = 1):
    if reps not in _NC_CACHE:
        _NC_CACHE[reps] = build_nc(reps)
    return _NC_CACHE[reps]


def kernel(x, wq, bq, wk, bk, wv, bv, wo):
    x, wq, bq, wk, bk, wv, bv, wo = (
        np.asarray(a, dtype=np.float32) for a in (x, wq, bq, wk, bk, wv, bv, wo)
    )
    nc = get_nc(1)
    in_maps = make_in_maps(x, wq, bq, wk, bk, wv, bv, wo)
    res = run_bass_kernel_spmd(nc, in_maps, core_ids=list(range(8)))
    out = np.zeros((B, T, H), np.float32)
    for core in range(8):
        b, _g = divmod(core, G)
        out[b] += res.results[core]["out"]
    # v-bias contribution: softmax rows sum to 1, so attn @ (1 x bv) = 1 x bv;
    # through the output projection that is repeat_kv(bv) @ wo added to every row.
    bv_rep = np.repeat(bv.reshape(NKV, D), HPG, axis=0).reshape(H)
    out += (bv_rep @ wo).reshape(1, 1, H)
    return out



# revision 3
# speedup vs baseline: 4.0439x; 4.0439x over previous
"""Causal group-query attention on 8 trn2 NeuronCores.

Sharding: 2 batches x 4 KV-head groups = 8 cores. Each core computes, for its
(batch b, group g): q/k/v projections for the group's 4 query heads + 1 KV
head, causal attention, and a partial output projection against the group's
512 rows of wo. Host sums the 4 group partials per batch (f16 partials).

On-chip layout (fp16 on the PE, fp32 PSUM accumulation):
  xT  [128(Hp), 16(Ho), T]   <- DMA-transpose of host-cast x16
  qT  [128(d), 4(h), T]      <- wq.T @ xT   (per-head slices)
  kT  [128(d), T]            <- wk.T @ xT
  v   [128(Ts), 4(tl), 128(d)] natural     <- xT.T @ wv (4 groups / psum bank)
  scoresT [128(Ts), 2, Tq]   = kT_slice.T @ qT_slice     (PSUM f32, 2 banks)
     + causal additive mask (-400) applied by a PE matmul (identity x cmask)
  probsT  = exp(scale*scoresT)  one ACT instr per 2-block pair -> fp16
  oT  [128(d), Tq] += v_s.T @ probsT ; den[128, Tq] += ones.T @ probsT
  out partial f16 = (oT/den).T @ wo_shard  (drains split DVE/ACT)

Differences vs the v1 baseline (2781554 ns): causal mask moved from DVE to a
PE accumulate-matmul, exp batched 2 score-blocks per ACT instr (halves ACT
instruction overhead), v-proj drains batched 4x, o-projection of chunk c-1
interleaved into attention of chunk c to fill ACT-bound PE gaps, f16 output
(halves write traffic), PSUM: 2 mm + 2x2 sc + 1 oT + 1 den banks.
"""

import numpy as np

import concourse.bass as bass
import concourse.mybir as mybir
import concourse.tile as tile
from concourse import bacc
from concourse.bass_utils import run_bass_kernel_spmd

# Problem shapes (hardcoded per contract)
B = 2
T = 2048
H = 2048
NH = 16
NKV = 4
D = 128  # head dim
G = NKV  # groups = cores per batch
HPG = NH // NKV  # 4 query heads per group
DQ = HPG * D  # 512 q dims per group
P = 128
KO = H // P  # 16 contraction subtiles
TCH = 512  # T chunk
NCH = T // TCH  # 4
NT = T // P  # 16
F16 = mybir.dt.float16
F32 = mybir.dt.float32
SCALE = float(1.0 / np.sqrt(D))
MASK_ADD = -400.0  # additive causal mask pre-scale; SCALE*400 ~ 35 -> exp ~ 0

AF = mybir.ActivationFunctionType
ALU = mybir.AluOpType


def build_nc(reps: int = 1):
    nc = bacc.Bacc(
        "TRN2",
        target_bir_lowering=False,
        debug=False,
        enable_asserts=False,
        num_devices=8,
    )
    x16 = nc.dram_tensor("x16", [T, H], F16, kind="ExternalInput").ap()
    wq16 = nc.dram_tensor("wq16", [P, KO, DQ], F16, kind="ExternalInput").ap()
    wk16 = nc.dram_tensor("wk16", [P, KO, D], F16, kind="ExternalInput").ap()
    wv16 = nc.dram_tensor("wv16", [P, KO, D], F16, kind="ExternalInput").ap()
    wo16 = nc.dram_tensor("wo16", [P, HPG, H], F16, kind="ExternalInput").ap()
    bq16 = nc.dram_tensor("bq16", [P, HPG], F16, kind="ExternalInput").ap()
    onem = nc.dram_tensor("onem", [P, P], F16, kind="ExternalInput").ap()
    id16m = nc.dram_tensor("id16m", [P, P], F16, kind="ExternalInput").ap()
    cmaskm = nc.dram_tensor("cmaskm", [P, 2 * P], F16, kind="ExternalInput").ap()
    out = nc.dram_tensor("out", [T, H], F16, kind="ExternalOutput").ap()

    with tile.TileContext(nc) as tc:
        with (
            tc.tile_pool(name="const", bufs=1) as cp,
            tc.tile_pool(name="pers", bufs=1) as pp,
            tc.tile_pool(name="probs", bufs=4) as prp,
            tc.tile_pool(name="bcast", bufs=2) as bcp,
            tc.tile_pool(name="outb", bufs=4) as obp,
            tc.tile_pool(name="mmps", bufs=2, space="PSUM") as mm_ps,
            tc.tile_pool(name="scps", bufs=2, space="PSUM") as sc_psp,
            tc.tile_pool(name="otps", bufs=1, space="PSUM") as ot_psp,
            tc.tile_pool(name="denps", bufs=1, space="PSUM") as den_psp,
        ):
            # ---- persistent SBUF residents ----
            wq_sb = cp.tile([P, KO, DQ], F16)
            wk_sb = cp.tile([P, KO, D], F16)
            wv_sb = cp.tile([P, KO, D], F16)
            wo_sb = cp.tile([P, HPG, H], F16)
            bq_sb = cp.tile([P, HPG], F16)
            onem_sb = cp.tile([P, P], F16)
            id16_sb = cp.tile([P, P], F16)
            cmask_sb = cp.tile([P, 2 * P], F16)

            xT_c = [pp.tile([P, KO, TCH], F16, name=f"xT{c}") for c in range(NCH)]
            qT_t = [
                [pp.tile([P, TCH], F16, name=f"qT{h}_{c}") for c in range(NCH)]
                for h in range(HPG)
            ]
            kT_t = [pp.tile([P, TCH], F16, name=f"kT{c}") for c in range(NCH)]
            v4_t = [pp.tile([P, 4, D], F16, name=f"v4_{c}") for c in range(NCH)]
            oT_t = [
                [pp.tile([P, TCH], F16, name=f"oT{h}_{c}") for c in range(NCH)]
                for h in range(HPG)
            ]

            def transpose_chunk(c):
                for tl in range(4):
                    tb = c * 4 + tl
                    nc.sync.dma_start_transpose(
                        xT_c[c][:, :, tl * P : (tl + 1) * P],
                        x16[tb * P : (tb + 1) * P, :],
                    )

            def projections(c):
                xc = xT_c[c]
                for m in range(HPG):
                    ps = mm_ps.tile([P, TCH], F32, name="mm", tag="mm")
                    for k in range(KO):
                        nc.tensor.matmul(
                            ps[:],
                            wq_sb[:, k, m * P : (m + 1) * P],
                            xc[:, k, :],
                            start=(k == 0),
                            stop=(k == KO - 1),
                        )
                    nc.vector.tensor_tensor(
                        qT_t[m][c][:],
                        ps[:],
                        bq_sb[:, m : m + 1].to_broadcast((P, TCH)),
                        ALU.add,
                    )
                ps = mm_ps.tile([P, TCH], F32, name="mm", tag="mm")
                for k in range(KO):
                    nc.tensor.matmul(
                        ps[:],
                        wk_sb[:, k, :],
                        xc[:, k, :],
                        start=(k == 0),
                        stop=(k == KO - 1),
                    )
                nc.vector.tensor_copy(kT_t[c][:], ps[:])
                # v: 4 sequential accumulation groups sharing one PSUM bank,
                # one batched drain
                psv = mm_ps.tile([P, 4, D], F32, name="mm", tag="mm")
                for tl in range(4):
                    for k in range(KO):
                        nc.tensor.matmul(
                            psv[:, tl, :],
                            xc[:, k, tl * P : (tl + 1) * P],
                            wv_sb[:, k, :],
                            start=(k == 0),
                            stop=(k == KO - 1),
                            skip_group_check=True,
                        )
                nc.vector.tensor_copy(v4_t[c][:], psv[:])

            def attention(c, h):
                oT_ps = ot_psp.tile([P, TCH], F32, name="oT_ps", tag="oT")
                den_ps = den_psp.tile([P, TCH], F32, name="den_ps", tag="den")
                nsb = 4 * c + 4
                for pr_i in range(nsb // 2):
                    j0 = 2 * pr_i
                    sc = sc_psp.tile([P, 2, TCH], F32, name="sc", tag="sc")
                    pr = prp.tile([P, 2, TCH], F16, name="pr", tag="pr")
                    los = [max(0, (j0 + idx) - 4 * c) * P for idx in range(2)]
                    lo_min = min(los)
                    for idx in range(2):
                        j = j0 + idx
                        jj = j - 4 * c
                        lo = los[idx]
                        diag = jj >= 0
                        # diag blocks compute scores from the pair's lo_min so
                        # the additive mask below lands on written PSUM; the
                        # masked sliver exps to ~0 and is never consumed.
                        nc.tensor.matmul(
                            sc[:, idx, (lo_min if diag else lo) :],
                            kT_t[j // 4][:, (j % 4) * P : (j % 4 + 1) * P],
                            qT_t[h][c][:, (lo_min if diag else lo) :],
                            start=True,
                            stop=(not diag),
                        )
                        if diag:
                            sw = lo - lo_min
                            nc.tensor.matmul(
                                sc[:, idx, lo - sw : lo + P],
                                id16_sb[:],
                                cmask_sb[:, P - sw : 2 * P],
                                start=False,
                                stop=True,
                            )
                    nc.scalar.activation(
                        pr[:, :, lo_min:], sc[:, :, lo_min:], AF.Exp, scale=SCALE
                    )
                    for idx in range(2):
                        j = j0 + idx
                        lo = los[idx]
                        nc.tensor.matmul(
                            den_ps[:, lo:],
                            onem_sb[:],
                            pr[:, idx, lo:],
                            start=(j == 0),
                            stop=(j == nsb - 1),
                            skip_group_check=True,
                        )
                        nc.tensor.matmul(
                            oT_ps[:, lo:],
                            v4_t[j // 4][:, j % 4, :],
                            pr[:, idx, lo:],
                            start=(j == 0),
                            stop=(j == nsb - 1),
                            skip_group_check=True,
                        )
                bc32 = bcp.tile([P, TCH], F32, name="bc32", tag="bc")
                nc.vector.reciprocal(bc32[:], den_ps[:])
                nc.vector.tensor_tensor(oT_t[h][c][:], oT_ps[:], bc32[:], ALU.mult)

            def oproj_tblock(cprev, tl):
                tb = cprev * 4 + tl
                for n in range(NCH):
                    ps = mm_ps.tile([P, TCH], F32, name="mm", tag="mm")
                    for hh in range(HPG):
                        nc.tensor.matmul(
                            ps[:],
                            oT_t[hh][cprev][:, tl * P : (tl + 1) * P],
                            wo_sb[:, hh, n * TCH : (n + 1) * TCH],
                            start=(hh == 0),
                            stop=(hh == HPG - 1),
                        )
                    ob = obp.tile([P, TCH], F16, name="ob", tag="ob")
                    if n % 2 == 0:
                        nc.vector.tensor_copy(ob[:], ps[:])
                    else:
                        nc.scalar.copy(ob[:], ps[:])
                    nc.sync.dma_start(
                        out[tb * P : (tb + 1) * P, n * TCH : (n + 1) * TCH],
                        ob[:],
                    )

            def body(first=False):
                if not first:
                    transpose_chunk(0)
                for c in range(NCH):
                    if c + 1 < NCH:
                        transpose_chunk(c + 1)
                    projections(c)
                    for h in range(HPG):
                        attention(c, h)
                        if c > 0:
                            oproj_tblock(c - 1, h)
                for tl in range(4):
                    oproj_tblock(NCH - 1, tl)

            transpose_chunk(0)
            nc.sync.dma_start(wq_sb[:], wq16)
            nc.sync.dma_start(wk_sb[:], wk16)
            nc.sync.dma_start(wv_sb[:], wv16)
            nc.sync.dma_start(bq_sb[:], bq16)
            nc.sync.dma_start(onem_sb[:], onem)
            nc.sync.dma_start(id16_sb[:], id16m)
            nc.sync.dma_start(cmask_sb[:], cmaskm)
            nc.sync.dma_start(wo_sb[:], wo16)
            if reps == 1:
                body(first=True)
            else:
                body(first=True)
                with tc.For_i(0, reps - 1, 1):
                    body()

    nc.compile()
    return nc


def make_in_maps(x, wq, bq, wk, bk, wv, bv, wo):
    # bk shifts every score in a query row equally (softmax-invariant) and is
    # dropped; bv passes through softmax as a constant row handled on host.
    del bk, bv
    f16 = np.float16
    cmask = np.full((P, 2 * P), MASK_ADD, f16)
    for s in range(P):
        cmask[s, P + s :] = 0.0  # keep q >= s in the diagonal block
    onem = np.ones((P, P), f16)
    id16 = np.eye(P, dtype=f16)
    in_maps = []
    for core in range(8):
        b, g = divmod(core, G)
        wq_s = wq[:, g * DQ : (g + 1) * DQ].astype(f16)
        wk_s = wk[:, g * D : (g + 1) * D].astype(f16)
        wv_s = wv[:, g * D : (g + 1) * D].astype(f16)
        wo_s = wo[g * DQ : (g + 1) * DQ, :].astype(f16)
        in_maps.append(
            {
                "x16": np.ascontiguousarray(x[b].astype(f16)),
                "wq16": np.ascontiguousarray(
                    wq_s.reshape(KO, P, DQ).transpose(1, 0, 2)
                ),
                "wk16": np.ascontiguousarray(wk_s.reshape(KO, P, D).transpose(1, 0, 2)),
                "wv16": np.ascontiguousarray(wv_s.reshape(KO, P, D).transpose(1, 0, 2)),
                "wo16": np.ascontiguousarray(
                    wo_s.reshape(HPG, P, H).transpose(1, 0, 2)
                ),
                "bq16": np.ascontiguousarray(
                    bq[g * DQ : (g + 1) * DQ].astype(f16).reshape(HPG, P).T
                ),
                "onem": onem,
                "id16m": id16,
                "cmaskm": cmask,
            }
        )
    return in_maps


_NC_CACHE = {}


def get_nc(reps: int = 1):
    if reps not in _NC_CACHE:
        _NC_CACHE[reps] = build_nc(reps)
    return _NC_CACHE[reps]


def kernel(x, wq, bq, wk, bk, wv, bv, wo):
    x, wq, bq, wk, bk, wv, bv, wo = (
        np.asarray(a, dtype=np.float32) for a in (x, wq, bq, wk, bk, wv, bv, wo)
    )
    nc = get_nc(1)
    in_maps = make_in_maps(x, wq, bq, wk, bk, wv, bv, wo)
    res = run_bass_kernel_spmd(nc, in_maps, core_ids=list(range(8)))
    out = np.zeros((B, T, H), np.float32)
    for core in range(8):
        b, _g = divmod(core, G)
        out[b] += res.results[core]["out"].astype(np.float32)
    # v-bias contribution: softmax rows sum to 1, so attn @ (1 x bv) = 1 x bv;
    # through the output projection that is repeat_kv(bv) @ wo added to every row.
    bv_rep = np.repeat(bv.reshape(NKV, D), HPG, axis=0).reshape(H)
    out += (bv_rep @ wo).reshape(1, 1, H)
    return out
